# revision 16
# baseline (speedup 1.0000x reference)
"""Trainium2 Bass kernel for nn_MHA_65429531787938.

MHA with a faithful-quirk softmax over dim=0 (the batch axis, B=2).
For B=2 the batch-softmax collapses to an elementwise sigmoid:
    attn0 = sigmoid((s0 - s1)/SCALE),  attn1 = 1 - attn0
and (1-A0) @ V1 = colsum(V1) - A0 @ V1, so a single attention matrix
serves both batches.

Sharding: tensor-parallel over the 16 heads -> 2 heads per core
(columns of w_q/w_k/w_v, rows of W_o). Each core consumes the full x
and produces a partial output (its heads' contribution to out = vals @ W_o);
the host sums the 8 partials.

Host-side prep (relayout/cast only, all math on device): x is
pre-transposed to x^T fp16 in chunk-major layout, weights pre-cast to
fp16, output written as bf16 partials. This removes the per-core PE
transposes + fp32->fp16 casts of the previous version and halves DMA.

Per-core pipeline:
  phase A: 4 chunks (b, half) of 1024 positions: q^T/k^T/v^T chains
           (8 fp16 matmuls each, N=1024, fp32 psum); q/k regrouped to
           batch-stacked [Q0;-Q1]/[K0;K1] layouts; v^T transposed via
           PE to natural V tiles (V1 negated).
  phase B: per (qc, head): d^T = K@Q in one N=1024 matmul per k-tile;
           A0^T = sigmoid(d^T/SCALE) on ACT (fp16); AV accumulation
           [V0|-V1]@A0^T + rank-1 colsum(V1) correction; sigmoids run
           ahead of AV bursts so ACT never gates the PE stream.
           Out-projection blocks of finished q-chunks are interleaved
           to fill PE slack; bf16 partial blocks DMA'd out.
"""

import numpy as np

import concourse.bacc as bacc
import concourse.mybir as mybir
import concourse.tile as tile
from concourse import bass_utils
from concourse.masks import make_identity

B, S, D, H = 2, 2048, 1024, 16
HD = 64
SCALE = float(D) ** 0.5
NCORES = 8
HPC = H // NCORES            # heads per core = 2
MS = HPC * HD                # per-core slice width = 128
P = 128
W = 1024                     # moving-operand width (16-bit max is 1024)
NT = S // P                  # k tiles per head = 16
DT16 = mybir.dt.float16
BF16 = mybir.dt.bfloat16
F32 = mybir.dt.float32

# chunk processing order: (b, half); c0,c1 cover qpos/kpos 0-1023 of both
# batches so attention on the first q-chunk can start after two chunks
CH = [(0, 0), (1, 0), (0, 1), (1, 1)]


def build():
    nc = bacc.Bacc("TRN2", target_bir_lowering=False, debug=False)

    # xt: [chunk, ktile, p, pos] = x^T fp16, chunk-major (host-prepped)
    xt_d = nc.dram_tensor("xt", [4, D // P, P, W], DT16, kind="ExternalInput").ap()
    wq_d = nc.dram_tensor("wq", [P, D // P, MS], DT16, kind="ExternalInput").ap()
    wk_d = nc.dram_tensor("wk", [P, D // P, MS], DT16, kind="ExternalInput").ap()
    wv_d = nc.dram_tensor("wv", [P, D // P, MS], DT16, kind="ExternalInput").ap()
    wo_d = nc.dram_tensor("wo", [MS, D], DT16, kind="ExternalInput").ap()
    # out: [block, qpos, d] with block = b*16 + qpos//128 (host reshapes)
    out_d = nc.dram_tensor("out", [B * S // P, P, D], BF16, kind="ExternalOutput").ap()

    with tile.TileContext(nc) as tc:
        with tc.tile_pool(name="persist", bufs=1) as pp, \
             tc.tile_pool(name="xtp", bufs=32) as xtp, \
             tc.tile_pool(name="atp", bufs=26) as atp, \
             tc.tile_pool(name="vtp", bufs=2) as vtp, \
             tc.tile_pool(name="otp", bufs=4) as otp, \
             tc.tile_pool(name="psA", bufs=1, space="PSUM") as psA:
            ident16 = pp.tile([P, P], DT16, name="ident16")
            make_identity(nc, ident16[:])
            ones_w = pp.tile([1, W], DT16)
            nc.vector.memset(ones_w[:], 1.0)
            ones128 = pp.tile([P, 1], DT16)
            nc.vector.memset(ones128[:], 1.0)

            # weights: fp16 direct loads (host pre-cast + pre-tiled), spread
            # across rings so the first x chunk isn't stuck behind them
            w_sb = {}
            for (name, dram), ring in zip(
                (("q", wq_d), ("k", wk_d), ("v", wv_d)),
                (nc.sync, nc.gpsimd, nc.scalar),
            ):
                t = pp.tile([P, D // P, MS], DT16, name=f"w{name}_sb")
                ring.dma_start(t[:], dram)
                w_sb[name] = t
            wo_sb = pp.tile([P, D], DT16)
            nc.scalar.dma_start(wo_sb[:], wo_d)

            # big persistent tensors
            qsb = pp.tile([P, HPC, S], DT16)     # [(b,hd), head, qpos], b1 negated
            ksb = pp.tile([P, HPC, S], DT16)     # [(b,hd), head, kpos]
            v_sb = pp.tile([P, NT, HPC, B, HD], DT16)  # [k, ktile, h, b, hd], b1 neg
            vals_sb = pp.tile([P, B, S], DT16)   # [(h,hd), batch, qpos]
            c1_sb = pp.tile([1, HPC, HD], DT16)  # +colsum(V1) per head

            # x^T chunk tiles: issue all DMAs up front (pool holds all 32)
            xts = {}
            for ci in range(4):
                for t in range(D // P):
                    xt = xtp.tile([P, W], DT16, tag="xt", name="xt")
                    ring = nc.sync if (ci * 8 + t) % 2 == 0 else nc.gpsimd
                    ring.dma_start(xt[:], xt_d[ci, t])
                    xts[(ci, t)] = xt

            at_tiles = {}

            def emit_chain(ci, dest, pool):
                """projection chain for chunk ci, dest in q/k/v + copies.

                For v, returns a closure emitting the PE transposes of v^T
                into natural V tiles — call it after queueing other PE work
                so the PE doesn't stall on the scalar-engine v^T copy."""
                b, half = CH[ci]
                ps = pool.tile([P, W], F32, tag="big", name="ps")
                # matmul output is capped at 512 fp32 (one PSUM bank): two
                # half-width accumulation groups fill the [P, 1024] tile
                for g in range(2):
                    gs = slice(g * 512, (g + 1) * 512)
                    for t in range(D // P):
                        nc.tensor.matmul(
                            ps[:, gs], w_sb[dest][:, t, :], xts[(ci, t)][:, gs],
                            start=(t == 0), stop=(t == D // P - 1),
                        )
                cols = slice(half * W, (half + 1) * W)
                if dest == "v":
                    vt = vtp.tile([P, W], DT16, tag="vt", name="vt")
                    nc.scalar.mul(vt[:], ps[:], -1.0 if b == 1 else 1.0)

                    def do_transposes():
                        # GPSIMD cannot read PSUM: natural-V copy on scalar
                        pvt = pool.tile([P, D // P, P], DT16, tag="big",
                                        name="pvt")
                        for blk in range(W // P):
                            nc.tensor.transpose(
                                pvt[:, blk, :], vt[:, blk * P:(blk + 1) * P],
                                ident16[:],
                            )
                        nc.scalar.copy(
                            v_sb[:, half * 8:(half + 1) * 8, :, b, :],
                            pvt[:].rearrange("p t (h d) -> p t h d", h=HPC),
                        )
                    return do_transposes
                dst = qsb if dest == "q" else ksb
                neg = dest == "q" and b == 1
                for h in range(HPC):
                    nc.vector.tensor_scalar_mul(
                        dst[b * HD:(b + 1) * HD, h, cols],
                        ps[h * HD:(h + 1) * HD, :],
                        -1.0 if neg else 1.0,
                    )
                return None

            def emit_score(h, c0, w, t, pool, tag):
                """scores for head h, qpos [c0, c0+w), k-tile t + sigmoid."""
                pd = pool.tile([P, w], F32, tag=tag, name="pd")
                for g in range(w // 512):
                    nc.tensor.matmul(
                        pd[:, g * 512:(g + 1) * 512],
                        ksb[:, h, t * P:(t + 1) * P],
                        qsb[:, h, c0 + g * 512:c0 + (g + 1) * 512],
                        start=True, stop=True,
                    )
                at = atp.tile([P, w], DT16, tag="at", name="at")
                nc.scalar.activation(
                    at[:], pd[:], mybir.ActivationFunctionType.Sigmoid,
                    scale=1.0 / SCALE,
                )
                at_tiles[(h, c0, t)] = at

            def emit_av(h, c0, t, pav):
                at = at_tiles.pop((h, c0, t))
                w = at.shape[-1]
                for g in range(w // 512):
                    gs = slice(g * 512, (g + 1) * 512)
                    nc.tensor.matmul(
                        pav[:, gs],
                        v_sb[:, t, h, :, :].rearrange("p b d -> p (b d)"),
                        at[:, gs],
                        start=(t == 0), stop=False,
                    )

            def emit_unit_end(h, c0, w, pav):
                # rank-1 correction (+colsum(V1) broadcast over qpos), then
                # regroup psum -> vals (PSUM reads must stay off GPSIMD)
                for g in range(w // 512):
                    nc.tensor.matmul(
                        pav[HD:2 * HD, g * 512:(g + 1) * 512],
                        c1_sb[:, h, :], ones_w[:, :512],
                        start=False, stop=True,
                    )
                for b in range(B):
                    nc.vector.tensor_scalar_mul(
                        vals_sb[h * HD:(h + 1) * HD, b, c0:c0 + w],
                        pav[b * HD:(b + 1) * HD, :], 1.0,
                    )

            ob_count = [0]

            def emit_outblock(b, si, pool, tail=False):
                ot = otp.tile([P, D], BF16, tag="ot", name="ot")
                for nch in range(2):
                    po = pool.tile([P, 512], F32, tag="po", name="po")
                    nc.tensor.matmul(
                        po[:], vals_sb[:, b, si * P:(si + 1) * P],
                        wo_sb[:, nch * 512:(nch + 1) * 512],
                        start=True, stop=True,
                    )
                    i = ob_count[0]
                    ob_count[0] += 1
                    if tail and i % 2 == 1:
                        nc.scalar.copy(ot[:, nch * 512:(nch + 1) * 512], po[:])
                    else:
                        nc.vector.tensor_copy(
                            ot[:, nch * 512:(nch + 1) * 512], po[:]
                        )
                ring = (nc.sync, nc.gpsimd, nc.scalar)[si % 3] if tail else \
                       (nc.sync if si % 2 == 0 else nc.gpsimd)
                ring.dma_start(out_d[b * (S // P) + si], ot[:])

            # ---------------- phase A head: chunks c0, c1 ----------------
            with tc.tile_pool(name="psB", bufs=3, space="PSUM") as psB:
                emit_chain(0, "q", psB)
                emit_chain(0, "k", psB)
                tr0 = emit_chain(0, "v", psB)
                emit_chain(1, "q", psB)
                tr0()
                emit_chain(1, "k", psB)
                tr1 = emit_chain(1, "v", psB)

                # ------- phase A tail interleaved with qc0 attention -------
                # scores for (0,h) t0..7 only need k-tiles 0-7 (chunks c0,c1)
                emit_chain(2, "q", psB)
                tr1()
                for t in (0, 1):
                    emit_score(0, 0, W, t, psB, "big")
                emit_chain(2, "k", psB)
                for t in (2, 3):
                    emit_score(0, 0, W, t, psB, "big")
                tr2 = emit_chain(2, "v", psB)
                for t in (4, 5):
                    emit_score(0, 0, W, t, psB, "big")
                emit_chain(3, "q", psB)
                tr2()
                for t in (6, 7):
                    emit_score(0, 0, W, t, psB, "big")
                emit_chain(3, "k", psB)
                for t in (0, 1, 2):
                    emit_score(1, 0, W, t, psB, "big")
                tr3 = emit_chain(3, "v", psB)
                for t in (3, 4):
                    emit_score(1, 0, W, t, psB, "big")
                tr3()
                emit_score(1, 0, W, 5, psB, "big")

                # colsum of V columns (all k); extract b=1 (stored negated)
                pc1 = psB.tile([1, HPC * B * HD], F32, tag="big", name="pc1")
                for t in range(NT):
                    nc.tensor.matmul(
                        pc1[:], ones128[:],
                        v_sb[:, t, :, :, :].rearrange("p h b d -> p (h b d)"),
                        start=(t == 0), stop=(t == NT - 1),
                    )
                nc.vector.tensor_scalar_mul(
                    c1_sb[:],
                    pc1[:].rearrange("p (h b d) -> p h b d", h=HPC, b=B)[:, :, 1, :],
                    -1.0,
                )

                # unit (h0, qc0): remaining scores + AV burst; rest of the
                # (h1, qc0) scores interleaved so ACT stays ahead of AV bursts
                pav = psA.tile([P, W], F32, tag="pav", name="pav")
                for t in range(8, 16):
                    emit_score(0, 0, W, t, psB, "big")
                    emit_av(0, 0, t - 8, pav)
                for t in (6, 7, 8):
                    emit_score(1, 0, W, t, psB, "big")
                s01 = iter(range(9, 16))
                for t in range(8, 16):
                    emit_av(0, 0, t, pav)
                    nt = next(s01, None)
                    if nt is not None:
                        emit_score(1, 0, W, nt, psB, "big")
                emit_unit_end(0, 0, W, pav)

                # unit (h1, qc0): all sigmoids already issued -> pure AV burst
                pav = psA.tile([P, W], F32, tag="pav", name="pav")
                for t in range(NT):
                    emit_av(1, 0, t, pav)
                emit_unit_end(1, 0, W, pav)

            # ---------------- phase B: qc1 + out-projection ----------------
            with tc.tile_pool(name="psC", bufs=2, space="PSUM") as psC:
                # unit (h0, qc1): streaming scores/AV + outproj(qc0) interleave
                pav = psA.tile([P, W], F32, tag="pav", name="pav")
                ob = iter([(b, si) for b in range(B) for si in range(8)])
                for t in range(NT):
                    emit_score(0, W, W, t, psC, "pd")
                    if t >= 1:
                        emit_av(0, W, t - 1, pav)
                    if t % 2 == 0:
                        nb = next(ob, None)
                        if nb is not None:
                            emit_outblock(nb[0], nb[1], psC)
                emit_av(0, W, NT - 1, pav)
                emit_unit_end(0, W, W, pav)

                # unit (h1, qc1): two 512-wide sub-units so the final
                # out-projection tail is halved
                for sub in range(2):
                    c0 = W + sub * 512
                    pav = psA.tile([P, 512], F32, tag="pav", name="pav")
                    for t in range(NT):
                        emit_score(1, c0, 512, t, psC, "pd")
                        if t >= 1:
                            emit_av(1, c0, t - 1, pav)
                        nb = next(ob, None)
                        if nb is not None:
                            emit_outblock(nb[0], nb[1], psC)
                        elif sub == 1 and t >= 2 and t % 2 == 0:
                            # sub 0's vals are done: run its out blocks now
                            si = 8 + (t - 2) // 2
                            if si <= 11:
                                emit_outblock(0, si, psC)
                                emit_outblock(1, si, psC)
                    emit_av(1, c0, NT - 1, pav)
                    emit_unit_end(1, c0, 512, pav)

                # tail: remaining out blocks (last sub-unit's qpos range)
                for si in range(12, 16):
                    for b in range(B):
                        emit_outblock(b, si, psC, tail=True)

    nc.compile()
    return nc


_NC = None


def _get_nc():
    global _NC
    if _NC is None:
        _NC = build()
    return _NC


def _prep_host(x, w_q, w_k, w_v, W_o):
    """Relayout/cast only — all arithmetic happens on device."""
    x = np.asarray(x, dtype=np.float32)
    # x^T fp16, chunk-major: [4][ktile][128][1024], chunk order per CH
    xt = np.empty((4, D // P, P, W), dtype=np.float16)
    for ci, (b, half) in enumerate(CH):
        xc = x[b, half * W:(half + 1) * W, :]        # [pos, d]
        xt[ci] = xc.T.reshape(D // P, P, W).astype(np.float16)
    wq = np.asarray(w_q, dtype=np.float16)
    wk = np.asarray(w_k, dtype=np.float16)
    wv = np.asarray(w_v, dtype=np.float16)
    wo = np.asarray(W_o, dtype=np.float16)
    return xt, wq, wk, wv, wo


def kernel(x, w_q, w_k, w_v, W_o, _trace=False):
    xt, wq, wk, wv, wo = _prep_host(x, w_q, w_k, w_v, W_o)

    nc = _get_nc()
    in_maps = []
    for i in range(NCORES):
        cs = slice(i * MS, (i + 1) * MS)
        # per-core weight slices, tiled for direct SBUF layout
        in_maps.append({
            "xt": xt,
            "wq": np.ascontiguousarray(
                wq[:, cs].reshape(D // P, P, MS).transpose(1, 0, 2)),
            "wk": np.ascontiguousarray(
                wk[:, cs].reshape(D // P, P, MS).transpose(1, 0, 2)),
            "wv": np.ascontiguousarray(
                wv[:, cs].reshape(D // P, P, MS).transpose(1, 0, 2)),
            "wo": np.ascontiguousarray(wo[cs, :]),
        })
    try:
        res = bass_utils.run_bass_kernel_spmd(
            nc, in_maps, core_ids=list(range(NCORES)), trace=_trace
        )
    except Exception:
        # transient NRT exec failures have been observed to succeed on retry
        res = bass_utils.run_bass_kernel_spmd(
            nc, in_maps, core_ids=list(range(NCORES)), trace=_trace
        )
    acc = np.zeros((B * S // P, P, D), dtype=np.float32)
    for i in range(NCORES):
        acc += np.asarray(res.results[i]["out"], dtype=np.float32)
    out = acc.reshape(B, S // P, P, D).reshape(B, S, D)
    if _trace:
        return out, res
    return out
